# revision 9
# baseline (speedup 1.0000x reference)
"""Trainium2 Bass kernel for nn_MoDE (mixture-of-depthwise-experts routing conv).

Problem: per-sample 5x5 'SAME' conv (64ch -> 64ch, 128x128 image, B=32) where
each sample's conv kernel is a gate-weighted mixture of 5 expert kernels
(conv5 / conv3 / conv1 / avg3-scaled / avg5-scaled), gates given by a per-task
linear+softmax gate.

Strategy:
 - Host (cheap, ~0.03% of FLOPs): compute gates, synthesize the per-sample
   5x5x64x64 kernel, zero-pad x to 132x132, and pack everything for the device.
 - Device (8 cores, pure data parallel, 4 samples/core): direct conv as 25
   shift-matmuls accumulating in PSUM.  Two samples are processed per matmul
   via a block-diagonal stationary operand:
       lhsT[128, 128] = [[Wa(k,l), 0], [0, Wb(k,l)]]   (rows = in-ch, cols = out-ch)
       rhs [128, 512] = [[xa_shift], [xb_shift]]       (4 output rows per tile)
   so each matmul does 2 samples x 64x64 channels for 512 pixels, giving
   12.5 effective shift-matmuls per sample instead of 25.

Raw-bass implementation (explicit semaphores): this walrus build cannot attach
sync waits to Matmult instructions (S3_LW has no wait slots), so all PE waits
are standalone wait_ge instructions.  TileContext is not used.
"""

import os
from contextlib import ExitStack

import numpy as np

B, CI, CO, H, W = 32, 64, 64, 128, 128
E, T = 5, 10
NCORES = 8
SPC = B // NCORES          # samples per core = 4
PAIRS = SPC // 2           # sample-pairs per core = 2
HP, WP = H + 4, W + 4      # padded image 132x132
NPIX = H * W
TILE_ROWS = 4              # output rows per PSUM tile
NT = H // TILE_ROWS        # 32 tiles per sample-pair
TN = TILE_ROWS * W         # 512 = one PSUM bank of fp32
NTILES = PAIRS * NT        # 64 tiles per core
PSBUF = 8                  # PSUM banks in rotation
OBUF = 16                  # SBUF output staging slots (2 batches)
BATCH = 8                  # out-DMA completion batch size

DTYPE = os.environ.get("MODE_KERNEL_DTYPE", "f32r")  # "f32r" | "bf16" | "f32"
XCHUNKS = int(os.environ.get("MODE_KERNEL_XCHUNKS", "4"))  # x DMA split per pair

LAST_RESULT = None         # BassKernelResults of the most recent run (for test.py)
_BASS_CACHE = {}


def _np_in_dtype():
    if DTYPE == "bf16":
        import ml_dtypes

        return ml_dtypes.bfloat16
    return np.float32


def _build_bass():
    import concourse.bass as bass
    import concourse.mybir as mybir

    f32 = mybir.dt.float32
    mdt = {
        "bf16": mybir.dt.bfloat16,
        "f32": mybir.dt.float32,
        "f32r": mybir.dt.float32r,
    }[DTYPE]

    nc = bass.Bass()
    xin = nc.dram_tensor("xin", [PAIRS, 128, HP, WP], mdt, kind="ExternalInput")
    win = nc.dram_tensor("win", [PAIRS, 128, 25, 128], mdt, kind="ExternalInput")
    yout = nc.dram_tensor("yout", [PAIRS, 128, NPIX], f32, kind="ExternalOutput")

    # Per-pair DMA plan: weights first (small), then x in XCHUNKS row-chunks so
    # the PE can start on early tiles while later rows are still in flight.
    # Chunk c covers padded rows [c*rows_per_chunk, ...); tile t needs padded
    # rows up to 4t+8, i.e. chunks up to ceil((4t+8)/rows_per_chunk).
    assert HP % XCHUNKS == 0
    RPC = HP // XCHUNKS  # padded rows per chunk
    # s_in increments per pair: 16 (weights) + 16*XCHUNKS (x chunks)
    PER_PAIR_IN = 16 * (XCHUNKS + 1)

    def chunks_needed(t):
        # x chunks that must have landed before computing tile t (rows 4t..4t+7)
        return min(XCHUNKS, -(-(4 * t + 8) // RPC))

    with ExitStack() as ctx:
        xt = [
            ctx.enter_context(nc.sbuf_tensor(f"xt{p}", [128, HP, WP], mdt))
            for p in range(PAIRS)
        ]
        wt = [
            ctx.enter_context(nc.sbuf_tensor(f"wt{p}", [128, 25, 128], mdt))
            for p in range(PAIRS)
        ]
        ot = ctx.enter_context(nc.sbuf_tensor("ot", [128, OBUF, TN], f32))
        ps = ctx.enter_context(nc.psum_tensor("ps", [128, PSBUF, TN], f32))
        s_pair = [
            ctx.enter_context(nc.semaphore(f"s_pair{p}")) for p in range(PAIRS)
        ]
        s_pe = ctx.enter_context(nc.semaphore("s_pe"))
        s_dve = ctx.enter_context(nc.semaphore("s_dve"))
        # Two alternating out-DMA sems (batches of OBUF tiles). DMA completions
        # reorder across queues, so waits must target "all issued so far done"
        # values only: batch k's stores land on sem k%2; before reusing the
        # slots in batch k we wait for ALL of batch k-2's stores.
        s_oab = [
            ctx.enter_context(nc.semaphore(f"s_oab{i}")) for i in range(2)
        ]
        block = ctx.enter_context(nc.Block())

        @block.sync
        def _(sync):
            for p in range(PAIRS):
                sync.dma_start(out=wt[p][:], in_=win[p]).then_inc(s_pair[p], 16)
                for c in range(XCHUNKS):
                    r0 = c * RPC
                    sync.dma_start(
                        out=xt[p][:, r0 : r0 + RPC, :], in_=xin[p, :, r0 : r0 + RPC, :]
                    ).then_inc(s_pair[p], 16)
            for j in range(NTILES):
                sync.wait_ge(s_dve, j + 1)
                p, t = divmod(j, NT)
                sync.dma_start(
                    out=yout[p, :, t * TN : (t + 1) * TN], in_=ot[:, j % OBUF, :]
                ).then_inc(s_oab[(j // BATCH) % 2], 16)

        @block.tensor
        def _(tensor):
            for j in range(NTILES):
                p, t = divmod(j, NT)
                tensor.wait_ge(s_pair[p], PER_PAIR_IN)
                if j >= PSBUF:
                    tensor.wait_ge(s_dve, j - PSBUF + 1)
                bank = ps[:, j % PSBUF, :]
                for kl in range(25):
                    k, l = divmod(kl, 5)
                    rhs = xt[p][:, 4 * t + k : 4 * t + k + TILE_ROWS, l : l + W]
                    mm = nc.tensor.matmul(
                        bank,
                        wt[p][:, kl, :],
                        rhs,
                        start=(kl == 0),
                        stop=(kl == 24),
                    )
                mm.then_inc(s_pe, 1)

        @block.vector
        def _(vector):
            for j in range(NTILES):
                k = j // BATCH
                vector.wait_ge(s_pe, j + 1)
                if k >= 2:
                    vector.wait_ge(s_oab[k % 2], 16 * BATCH * (k // 2))
                nc.vector.tensor_copy(ot[:, j % OBUF, :], ps[:, j % PSBUF, :]).then_inc(
                    s_dve, 1
                )

    return nc


def _get_bass():
    key = (DTYPE, XCHUNKS)
    if key not in _BASS_CACHE:
        _BASS_CACHE[key] = _build_bass()
    return _BASS_CACHE[key]


def _host_prep(x, task_id, gate_w, gate_b, w5, w3, w1, wavg3, wavg5):
    """Gate + kernel synthesis + padding/packing. Returns (xp_pairs, wl_pairs)."""
    # Gate: logits[b, e*CO+o] = gate_w[e*CO+o, task_b] + gate_b
    logits = gate_w[:, task_id].T + gate_b[None, :]          # [B, E*CO]
    gl = logits.reshape(B, E, CO)
    gl = gl - gl.max(axis=1, keepdims=True)
    ge = np.exp(gl)
    g = ge / ge.sum(axis=1, keepdims=True)                   # [B, E, CO]

    # Expert bank [E, O, I, 5, 5], all as 5x5 kernels
    bank = np.zeros((E, CO, CI, 5, 5), np.float32)
    bank[0] = w5
    bank[1, :, :, 1:4, 1:4] = w3
    bank[2, :, :, 2, 2] = w1[:, :, 0, 0]
    bank[3, :, :, 1:4, 1:4] = (wavg3[:, :, 0, 0] / 9.0)[:, :, None, None]
    bank[4] = (wavg5[:, :, 0, 0] / 25.0)[:, :, None, None]

    # Per-sample synthesized kernels [B, O, I, 5, 5]
    wsyn = np.einsum("beo,eoikl->boikl", g, bank, optimize=True).astype(np.float32)

    idt = _np_in_dtype()
    # Block-diagonal stationary layout [B/2, 128(i), 25(kl), 128(o)]
    wsynT = np.transpose(wsyn, (0, 2, 3, 4, 1)).reshape(B, CI, 25, CO)  # [b,i,kl,o]
    wl = np.zeros((B // 2, 128, 25, 128), idt)
    wl[:, 0:CI, :, 0:CO] = wsynT[0::2]
    wl[:, CI:128, :, CO:128] = wsynT[1::2]

    # Zero-padded, pair-stacked x [B/2, 128(ch), 132, 132]
    xp = np.zeros((B // 2, 128, HP, WP), idt)
    xp[:, 0:CI, 2 : 2 + H, 2 : 2 + W] = x[0::2]
    xp[:, CI:128, 2 : 2 + H, 2 : 2 + W] = x[1::2]
    return xp, wl


def kernel(**inputs):
    global LAST_RESULT
    from concourse.bass_utils import run_bass_kernel_spmd

    x = np.asarray(inputs["x"], np.float32)
    task_id = np.asarray(inputs["task_id"])
    gate_w = np.asarray(inputs["gate_w"], np.float32)
    gate_b = np.asarray(inputs["gate_b"], np.float32)
    w5 = np.asarray(inputs["w5"], np.float32)
    w3 = np.asarray(inputs["w3"], np.float32)
    w1 = np.asarray(inputs["w1"], np.float32)
    wavg3 = np.asarray(inputs["wavg3"], np.float32)
    wavg5 = np.asarray(inputs["wavg5"], np.float32)

    xp, wl = _host_prep(x, task_id, gate_w, gate_b, w5, w3, w1, wavg3, wavg5)

    in_maps = [
        {
            "xin": np.ascontiguousarray(xp[c * PAIRS : (c + 1) * PAIRS]),
            "win": np.ascontiguousarray(wl[c * PAIRS : (c + 1) * PAIRS]),
        }
        for c in range(NCORES)
    ]

    nc = _get_bass()
    res = run_bass_kernel_spmd(nc, in_maps, list(range(NCORES)))
    LAST_RESULT = res

    y = np.empty((B, CO, H, W), np.float32)
    for c in range(NCORES):
        out = res.results[c]["yout"]  # [PAIRS, 128, NPIX]
        for p in range(PAIRS):
            b = c * SPC + 2 * p
            y[b] = out[p, 0:CO].reshape(CO, H, W)
            y[b + 1] = out[p, CO:128].reshape(CO, H, W)
    return y, task_id
